# revision 6
# baseline (speedup 1.0000x reference)
"""TRN2 Bass/Tile kernel for nn_Loss_58317065945194.

Loss: per-sample EMD with r=2 over C=10 channels:
    d = p - q                       # [B, C]
    S = cumsum(d, axis=1)           # per-sample prefix sums
    per_sample = sqrt(mean(S**2))   # [B]
    out = mean(per_sample)          # scalar

Strategy (pure data parallel, 8 cores):
  - Shard B across 8 cores; per core reshape the [Bs, 10] shard to
    [128 partitions, 20480] (each partition holds 2048 whole samples,
    10 contiguous values each). Inputs are cast to fp16 host-side
    (halves HBM traffic; scan state stays fp32 internally).
  - Per chunk of W samples/partition:
      * GpSimd:  d = p - q                          (fp16 tensor_tensor)
      * Vector:  c = segmented-cumsum(d) via tensor_tensor_scan with a
                 0/1 mask that resets state at each sample start:
                 state = (mask[t] * state) + d[t]   (state kept fp32)
      * Scalar:  sq = c^2  (in place)
      * Vector:  U[g] = sum_j sq[g, j]   (3D AP, reduce axis=X)
      * Scalar:  loss = sqrt(U / C), accum_out -> per-chunk column
  - Each core returns [128, NCHUNK] fp32 partial sums of per-sample
    losses; the host sums all partials and divides by B (replaces the
    all-reduce).
"""

import sys

import numpy as np

if "/opt/trn_rl_repo" not in sys.path:
    sys.path.insert(0, "/opt/trn_rl_repo")

N_CORES = 8
B, C = 2097152, 10
BS = B // N_CORES        # samples per core shard
P = 128                  # SBUF partitions
FPP = BS * C // P        # elems per partition (20480)
W = 256                  # samples per chunk per partition
CW = W * C               # chunk free width (2560)
NCHUNK = FPP // CW       # chunks per core (8)

_cache = {}


def _build_program():
    import concourse.tile as tile
    from concourse import bacc, mybir

    f32, f16 = mybir.dt.float32, mybir.dt.float16
    Alu = mybir.AluOpType
    Act = mybir.ActivationFunctionType

    nc = bacc.Bacc(
        "TRN2", target_bir_lowering=False, debug=False, num_devices=N_CORES
    )
    p_d = nc.dram_tensor("p", [P, FPP], f16, kind="ExternalInput").ap()
    q_d = nc.dram_tensor("q", [P, FPP], f16, kind="ExternalInput").ap()
    o_d = nc.dram_tensor("partial", [P, NCHUNK], f32, kind="ExternalOutput").ap()

    with tile.TileContext(nc) as tc:
        with (
            tc.tile_pool(name="io", bufs=3) as io,
            tc.tile_pool(name="work", bufs=3) as work,
            tc.tile_pool(name="small", bufs=2) as small,
            tc.tile_pool(name="accp", bufs=1) as accp,
        ):
            acc = accp.tile([P, NCHUNK], f32)
            for ci in range(NCHUNK):
                pt = io.tile([P, CW], f16, tag="p")
                qt = io.tile([P, CW], f16, tag="q")
                nc.sync.dma_start(pt[:], p_d[:, ci * CW : (ci + 1) * CW])
                nc.sync.dma_start(qt[:], q_d[:, ci * CW : (ci + 1) * CW])

                # fused subtract + running prefix sum on Vector:
                # S[1+t] = (p[t] + state) - q[t]; S[0] = 0 (memset).
                # S crosses sample boundaries; fixed up below.
                S = work.tile([P, CW + 1], f16, tag="S")
                nc.gpsimd.memset(S[:, 0:1], 0.0)
                nc.vector.tensor_tensor_scan(
                    S[:, 1:], pt[:], qt[:], 0.0, Alu.add, Alu.subtract
                )

                # per-sample prefix sums on GpSimd:
                # c[g, j] = S[1 + 10g + j] - S[10g]  (broadcast subtract)
                s3 = S[:, 1:].rearrange("p (w c) -> p w c", c=C)
                b3 = S[:, 0:CW:C].unsqueeze(2).broadcast_to((P, W, C))
                cs = work.tile([P, CW], f16, tag="cs")
                cs3 = cs[:].rearrange("p (w c) -> p w c", c=C)
                nc.gpsimd.tensor_tensor(cs3, s3, b3, Alu.subtract)

                # square in place on Scalar engine
                nc.scalar.activation(cs[:], cs[:], Act.Square)

                # U[g] = sum_j c[g, j]^2
                cs3 = cs[:].rearrange("p (w c) -> p w c", c=C)
                U = small.tile([P, W], f32, tag="U")
                nc.vector.tensor_reduce(
                    U[:], cs3, axis=mybir.AxisListType.X, op=Alu.add
                )

                # loss[g] = sqrt(U[g] / C); acc[:, ci] = sum_g loss[g]
                lt = small.tile([P, W], f32, tag="loss")
                nc.scalar.activation(
                    lt[:], U[:], Act.Sqrt, scale=1.0 / C,
                    accum_out=acc[:, ci : ci + 1],
                )
            nc.sync.dma_start(o_d[:], acc[:])
    nc.compile()
    return nc


def _make_in_maps(p, q):
    p = np.asarray(p, dtype=np.float32).reshape(B, C).astype(np.float16)
    q = np.asarray(q, dtype=np.float32).reshape(B, C).astype(np.float16)
    in_maps = []
    for i in range(N_CORES):
        in_maps.append(
            {
                "p": np.ascontiguousarray(p[i * BS : (i + 1) * BS]).reshape(P, FPP),
                "q": np.ascontiguousarray(q[i * BS : (i + 1) * BS]).reshape(P, FPP),
            }
        )
    return in_maps


def kernel(p, q, r):
    assert int(r) == 2, f"kernel specialized for r=2, got {r}"
    if "nc" not in _cache:
        _cache["nc"] = _build_program()
    nc = _cache["nc"]

    in_maps = _make_in_maps(p, q)

    from concourse.bass_utils import run_bass_kernel_spmd

    res = run_bass_kernel_spmd(nc, in_maps, list(range(N_CORES)))
    total = 0.0
    for r_ in res.results:
        total += r_["partial"].astype(np.float64).sum()
    return np.float32(total / B)


# revision 7
# speedup vs baseline: 1.1101x; 1.1101x over previous
"""TRN2 Bass/Tile kernel for nn_Loss_58317065945194.

Loss: per-sample EMD with r=2 over C=10 channels:
    d = p - q                       # [B, C]
    S = cumsum(d, axis=1)           # per-sample prefix sums
    per_sample = sqrt(mean(S**2))   # [B]
    out = mean(per_sample)          # scalar

Strategy (pure data parallel, 8 cores):
  - Shard B across 8 cores; per core reshape the [Bs, 10] shard to
    [128 partitions, 20480] (each partition holds 2048 whole samples,
    10 contiguous values each). Inputs are cast to fp16 host-side
    (halves HBM traffic; scan state stays fp32 internally).
  - Per chunk of W samples/partition:
      * GpSimd:  d = p - q                          (fp16 tensor_tensor)
      * Vector:  c = segmented-cumsum(d) via tensor_tensor_scan with a
                 0/1 mask that resets state at each sample start:
                 state = (mask[t] * state) + d[t]   (state kept fp32)
      * Scalar:  sq = c^2  (in place)
      * Vector:  U[g] = sum_j sq[g, j]   (3D AP, reduce axis=X)
      * Scalar:  loss = sqrt(U / C), accum_out -> per-chunk column
  - Each core returns [128, NCHUNK] fp32 partial sums of per-sample
    losses; the host sums all partials and divides by B (replaces the
    all-reduce).
"""

import sys

import numpy as np

if "/opt/trn_rl_repo" not in sys.path:
    sys.path.insert(0, "/opt/trn_rl_repo")

N_CORES = 8
B, C = 2097152, 10
BS = B // N_CORES        # samples per core shard
P = 128                  # SBUF partitions
FPP = BS * C // P        # elems per partition (20480)
W = 256                  # samples per chunk per partition
CW = W * C               # chunk free width (2560)
NCHUNK = FPP // CW       # chunks per core (8)

_cache = {}


def _build_program():
    import concourse.tile as tile
    from concourse import bacc, mybir

    f32, f16 = mybir.dt.float32, mybir.dt.float16
    Alu = mybir.AluOpType
    Act = mybir.ActivationFunctionType

    nc = bacc.Bacc(
        "TRN2", target_bir_lowering=False, debug=False, num_devices=N_CORES
    )
    p_d = nc.dram_tensor("p", [P, FPP], f16, kind="ExternalInput").ap()
    q_d = nc.dram_tensor("q", [P, FPP], f16, kind="ExternalInput").ap()
    o_d = nc.dram_tensor("partial", [P, NCHUNK], f32, kind="ExternalOutput").ap()

    with tile.TileContext(nc) as tc:
        with (
            tc.tile_pool(name="io", bufs=4) as io,
            tc.tile_pool(name="work", bufs=4) as work,
            tc.tile_pool(name="small", bufs=2) as small,
            tc.tile_pool(name="accp", bufs=1) as accp,
        ):
            acc = accp.tile([P, NCHUNK], f32)
            for ci in range(NCHUNK):
                pt = io.tile([P, CW], f16, tag="p")
                qt = io.tile([P, CW], f16, tag="q")
                nc.sync.dma_start(pt[:], p_d[:, ci * CW : (ci + 1) * CW])
                nc.sync.dma_start(qt[:], q_d[:, ci * CW : (ci + 1) * CW])

                # fused subtract + running prefix sum on Vector:
                # S[8+t] = (p[t] + state) - q[t]; S[7] = 0 (memset).
                # Scan output starts at offset 8 (16B) to keep it aligned.
                # S crosses sample boundaries; fixed up below.
                S = work.tile([P, CW + 8], f16, tag="S")
                nc.gpsimd.memset(S[:, 7:8], 0.0)
                nc.vector.tensor_tensor_scan(
                    S[:, 8:], pt[:], qt[:], 0.0, Alu.add, Alu.subtract
                )

                # per-sample prefix sums: c[g, j] = S[8+10g+j] - S[8+10g-1]
                # (broadcast subtract; alternate engines to balance load)
                s3 = S[:, 8:].rearrange("p (w c) -> p w c", c=C)
                b3 = S[:, 7 : 7 + CW : C].unsqueeze(2).broadcast_to((P, W, C))
                cs = work.tile([P, CW], f16, tag="cs")
                cs3 = cs[:].rearrange("p (w c) -> p w c", c=C)
                eng = nc.gpsimd if ci % 2 == 0 else nc.vector
                eng.tensor_tensor(cs3, s3, b3, Alu.subtract)

                # square in place on Scalar engine
                nc.scalar.activation(cs[:], cs[:], Act.Square)

                # U[g] = sum_j c[g, j]^2
                cs3 = cs[:].rearrange("p (w c) -> p w c", c=C)
                U = small.tile([P, W], f32, tag="U")
                nc.vector.tensor_reduce(
                    U[:], cs3, axis=mybir.AxisListType.X, op=Alu.add
                )

                # loss[g] = sqrt(U[g] / C); acc[:, ci] = sum_g loss[g]
                lt = small.tile([P, W], f32, tag="loss")
                nc.scalar.activation(
                    lt[:], U[:], Act.Sqrt, scale=1.0 / C,
                    accum_out=acc[:, ci : ci + 1],
                )
            nc.sync.dma_start(o_d[:], acc[:])
    nc.compile()
    return nc


def _make_in_maps(p, q):
    p = np.asarray(p, dtype=np.float32).reshape(B, C).astype(np.float16)
    q = np.asarray(q, dtype=np.float32).reshape(B, C).astype(np.float16)
    in_maps = []
    for i in range(N_CORES):
        in_maps.append(
            {
                "p": np.ascontiguousarray(p[i * BS : (i + 1) * BS]).reshape(P, FPP),
                "q": np.ascontiguousarray(q[i * BS : (i + 1) * BS]).reshape(P, FPP),
            }
        )
    return in_maps


def kernel(p, q, r):
    assert int(r) == 2, f"kernel specialized for r=2, got {r}"
    if "nc" not in _cache:
        _cache["nc"] = _build_program()
    nc = _cache["nc"]

    in_maps = _make_in_maps(p, q)

    from concourse.bass_utils import run_bass_kernel_spmd

    res = run_bass_kernel_spmd(nc, in_maps, list(range(N_CORES)))
    total = 0.0
    for r_ in res.results:
        total += r_["partial"].astype(np.float64).sum()
    return np.float32(total / B)
